# revision 34
# baseline (speedup 1.0000x reference)
"""Average-Precision (histogram binning) kernel for 8 Trainium2 NeuronCores.

Reference semantics (C=2 classes, T=10 thresholds):
  s = y_pred[:, 1, ...] flattened, y = y_true flattened
  per threshold t: fp = #(y==0 & s>t), tp = #(y==1 & s>t), P = #(y==1)
  AP = trapezoid area over (recall, precision) with endpoint padding.

Device strategy (data-parallel, 1.57M voxels per core):
  Host re-encodes each (s, y) pair losslessly into one fp16 value
  v = (1-2y) * fp16(s) (label in the sign bit, score in the magnitude),
  so all 21 statistics are single-comparison counts on v:
    fp[t] = #(v > t), tp[t] = #(v < -t), P = #(v < 0).
  fp16(s) only moves each effective threshold by <= half an ulp,
  identically for tp and fp => AP error ~1e-3 << 2e-2 gate.

  Three counting lanes, balanced to the measured engine rates
  (DVE 4x tensor_scalar ~0.27ns/elem, ACT 1x ~0.85ns/elem,
   PE ones-matmul reduce ~0.5ns/elem):
   - R1: DVE tensor_scalar is_gt/is_lt makes an fp16 {0,1} mask; PE
     reduces it with a one-hot fp16 stationary into an accumulating
     PSUM row (exact integer counts).
   - R8: same mask, then one DVE tensor_tensor fold (adds the two
     halves, values {0,1,2}) so PE only reduces half the columns.
   - ACT: Sign(+-(v-theta)) with fused accum_out (sign-sum decode).
  Per-tile ACT accum columns are partition-reduced by one tiny matmul
  per tile into an accumulating [1, W] PSUM row. Host sums the 8
  per-core results, decodes, and applies the AP formula.
"""

import sys

import numpy as np

for _p in ("/opt/trn_rl_repo", "/opt/pypackages"):
    if _p not in sys.path:
        sys.path.append(_p)

NUM_CORES = 8
P = 128
FTOT = 12288  # per-core columns: 8 * 128 * 12288 = 12,582,912 voxels
EPS = 1e-7
T = 10
NB = 21  # 10 fp + 10 tp + P

CFG = {
    "tile_sizes": [2048, 6144, 4096],
    "act_idx": [15, 16, 17, 18, 19, 20],  # ACT sign lane boundaries
    "r8_idx": [10, 11, 12, 13, 14],       # DVE mask + fold -> PE half
    # remaining boundaries go to R1 (DVE mask -> PE full reduce)
    "io_bufs": 2,
    "msk_bufs": 7,
    "fld_bufs": 2,
    "sg_bufs": 2,
    "onehot_w": 16,
}


def _boundaries(thresholds):
    """21 boundaries on v: 0..9 fp[t] (gt, +t), 10..19 tp[t] (lt, -t),
    20 P (lt, 0)."""
    th = np.asarray(thresholds, np.float64)
    bounds = [("gt", float(t)) for t in th]
    bounds += [("lt", -float(t)) for t in th]
    bounds += [("lt", 0.0)]
    return bounds


def _nudge_off_grid(theta):
    """Shift theta off the fp16 grid so Sign(v - theta) never sees 0.
    Any shift smaller than half the local fp16 gap leaves all strict
    comparison counts unchanged."""
    t32 = np.float32(theta)
    if t32 == 0.0:
        return float(t32)
    if np.float32(np.float16(t32)) == t32:
        t32 = np.float32(t32 * (1.0 + 2.0 ** -12))
    return float(t32)


def _assignment():
    act_idx = CFG["act_idx"]
    r8_idx = CFG["r8_idx"]
    r1_idx = [b for b in range(NB) if b not in act_idx and b not in r8_idx]
    return r1_idx, r8_idx, act_idx


def _build(thresholds):
    from concourse import bacc, mybir
    from concourse import tile

    dt = mybir.dt
    Alu = mybir.AluOpType
    AF = mybir.ActivationFunctionType

    bounds = _boundaries(thresholds)
    sizes = CFG["tile_sizes"]
    assert sum(sizes) == FTOT
    # R8 folds reduce FT//2 columns in 512-wide PE chunks: every tile
    # size must be a multiple of 1024 or columns get silently dropped
    assert all(s % 1024 == 0 for s in sizes), sizes
    NT = len(sizes)

    r1_idx, r8_idx, act_idx = _assignment()
    n_pe = len(r1_idx) + len(r8_idx)  # PSUM rows used
    n_act = len(act_idx)
    OW = CFG["onehot_w"]
    assert n_pe <= OW

    nc = bacc.Bacc(
        "TRN2", target_bir_lowering=False, debug=False, num_devices=NUM_CORES
    )
    v_exts = [
        nc.dram_tensor(f"v{j}", [P, sizes[j]], dt.float16,
                       kind="ExternalInput")
        for j in range(NT)
    ]
    # single merged output: rows 0..OW-1 col0 = PE counts, row OW = sign sums
    cnt_ext = nc.dram_tensor("cnt", [33, max(n_act, 1)], dt.float32,
                             kind="ExternalOutput")

    with tile.TileContext(nc) as tc:
        with (
            tc.tile_pool(name="io", bufs=CFG["io_bufs"]) as io_pool,
            tc.tile_pool(name="msk", bufs=CFG["msk_bufs"]) as msk_pool,
            tc.tile_pool(name="fld", bufs=CFG["fld_bufs"]) as fld_pool,
            tc.tile_pool(name="sg", bufs=CFG["sg_bufs"]) as sg_pool,
            tc.tile_pool(name="acc", bufs=2) as acc_pool,
            tc.tile_pool(name="fin", bufs=1) as fin_pool,
            tc.tile_pool(name="cst", bufs=1) as cst_pool,
            tc.tile_pool(name="psA", bufs=1, space="PSUM") as psA_pool,
            tc.tile_pool(name="psB", bufs=1, space="PSUM") as psB_pool,
        ):
            # ---- input DMAs first: get bytes moving before const setup.
            # All on the Sync HWDGE ring: serialized transfers give tile 0
            # the full bandwidth, minimizing time-to-first-compute (issuing
            # tile 0 concurrently from the Scalar sequencer was measured to
            # split bandwidth and delay tile 0 by ~3us). Each tile is its
            # own contiguous DRAM tensor so the transfer is fully dense.
            v_tiles = []
            for j in range(NT):
                FT = sizes[j]
                v_t = io_pool.tile([P, FT], dt.float16, tag="v", name=f"v_{j}")
                nc.sync.dma_start(out=v_t[:], in_=v_exts[j][:, :])
                v_tiles.append(v_t)

            # ---- constants ----
            # one-hot stationary blocks: block i is [P, OW] with column i
            # all-ones, so PE lands boundary i's mask-sum in PSUM row i.
            oh = cst_pool.tile([P, n_pe * OW], dt.float16, name="oh")
            nc.vector.memset(oh[:], 0.0)
            for i in range(n_pe):
                nc.vector.memset(oh[:, i * OW + i : i * OW + i + 1], 1.0)
            ones_f32 = cst_pool.tile([P, 1], dt.float32, name="ones_f32")
            nc.vector.memset(ones_f32[:], 1.0)
            act_bias = []
            for i, b in enumerate(act_idx):
                kind, thr = bounds[b]
                theta = _nudge_off_grid(thr)
                bias = cst_pool.tile([P, 1], dt.float32, name=f"abias_{i}")
                # gt: sign(v - theta); lt: sign(theta - v)
                nc.vector.memset(bias[:], -theta if kind == "gt" else theta)
                act_bias.append(bias)

            # warm the Sign table set while the first DMA is in flight so
            # the ~2.7us ACT_TABLE_LOAD+drain stays off the ACT lane
            warm = cst_pool.tile([P, 1], dt.float16, name="warm")
            nc.scalar.activation(out=warm[:], in_=ones_f32[:], func=AF.Sign,
                                 bias=act_bias[0][:], scale=1.0)

            ps_te = psA_pool.tile([OW, 512], dt.float32, name="ps_te")
            ps_row = psB_pool.tile([1, n_act], dt.float32, name="ps_row")
            first_mm = [True]

            acc_tiles = []
            for j in range(NT):
                FT = sizes[j]
                last_tile = j == NT - 1
                v_t = v_tiles[j]

                # ---- ACT lane: Sign + fused accum ----
                acc_t = acc_pool.tile([P, n_act], dt.float32, tag="acc",
                                      name=f"acc_{j}")
                acc_tiles.append(acc_t)
                for i, b in enumerate(act_idx):
                    kind, _ = bounds[b]
                    scl = 1.0 if kind == "gt" else -1.0
                    sg = sg_pool.tile([P, FT], dt.float16, tag="sg",
                                      name=f"sg_{j}_{i}")
                    nc.scalar.activation(out=sg[:], in_=v_t[:], func=AF.Sign,
                                         bias=act_bias[i][:], scale=scl,
                                         accum_out=acc_t[:, i:i + 1])

                # ---- R1/R8 lanes interleaved (smooth PE feed):
                # R1: DVE mask -> PE full-width reduce
                # R8: DVE mask -> fold halves -> PE half reduce
                seq = []
                it1, it8 = iter(enumerate(r1_idx)), iter(enumerate(r8_idx))
                for t in range(max(len(r1_idx), len(r8_idx))):
                    for it, lane in ((it1, "r1"), (it1, "r1"), (it8, "r8")):
                        nxt = next(it, None)
                        if nxt is not None:
                            seq.append((lane, nxt[0], nxt[1]))
                n_seq = len(seq)
                for si, (lane, k, b) in enumerate(seq):
                    # previous tile's sign-sum partition-reduce lands a few
                    # MMs into this tile, so PE never stalls waiting on ACT
                    # at the tile boundary
                    if j > 0 and si == 2:
                        nc.tensor.matmul(ps_row[:], ones_f32[:],
                                         acc_tiles[j - 1][:],
                                         start=(j == 1), stop=False)
                    kind, thr = bounds[b]
                    op = Alu.is_gt if kind == "gt" else Alu.is_lt
                    mk = msk_pool.tile([P, FT], dt.float16, tag="mk",
                                       name=f"mk_{j}_{lane}_{k}")
                    nc.vector.tensor_scalar(out=mk[:], in0=v_t[:],
                                            scalar1=thr, scalar2=None, op0=op)
                    # fractional rebalance hook: folding an R1 boundary on a
                    # subset of tiles lands in the same PSUM row (counts add
                    # exactly); measured engine rates put the optimum at the
                    # plain 10/5 split, so no override is active
                    fold = lane == "r8"
                    if not fold:
                        red, kk = mk, k
                    else:
                        H = FT // 2
                        fd = fld_pool.tile([P, H], dt.float16, tag="fd",
                                           name=f"fd_{j}_{lane}_{k}")
                        nc.vector.tensor_tensor(out=fd[:], in0=mk[:, 0:H],
                                                in1=mk[:, H:FT], op=Alu.add)
                        kk = k if lane == "r1" else len(r1_idx) + k
                        red = fd
                    W = FT if not fold else FT // 2
                    for c in range(W // 512):
                        nc.tensor.matmul(
                            ps_te[:],
                            oh[:, kk * OW : kk * OW + OW],
                            red[:, c * 512:(c + 1) * 512],
                            start=first_mm[0],
                            stop=(last_tile and si == n_seq - 1
                                  and c == W // 512 - 1),
                        )
                        first_mm[0] = False

            # ---- last tile's sign-sum partition-reduce ----
            nc.tensor.matmul(ps_row[:], ones_f32[:], acc_tiles[NT - 1][:],
                             start=False, stop=True)

            # ---- finalize (reduce/copy straight from PSUM, one out DMA) ----
            fin = fin_pool.tile([33, max(n_act, 1)], dt.float32, name="fin")
            nc.vector.tensor_reduce(out=fin[0:OW, 0:1], in_=ps_te[:],
                                    axis=mybir.AxisListType.X, op=Alu.add)
            nc.vector.tensor_copy(fin[32:33, 0:n_act], ps_row[:])
            nc.sync.dma_start(out=cnt_ext[:], in_=fin[:])

    nc.compile()
    return nc


def _prepare_inputs(y_pred, y_true):
    """v = (1-2y) * fp16(s): lossless per-voxel re-encode of (s, y).
    Split per compute tile into contiguous blocks for dense DMA."""
    s = np.asarray(y_pred)[:, 1].reshape(-1).astype(np.float16)
    y = np.asarray(y_true).reshape(-1)
    v = np.where(y == 0, s, -s)
    n = v.size
    assert n == NUM_CORES * P * FTOT, n
    v_sh = v.reshape(NUM_CORES, P, FTOT)
    sizes = CFG["tile_sizes"]
    maps = []
    for i in range(NUM_CORES):
        m, col0 = {}, 0
        for j, ft in enumerate(sizes):
            m[f"v{j}"] = np.ascontiguousarray(v_sh[i, :, col0:col0 + ft])
            col0 += ft
        maps.append(m)
    return maps


def _decode_counts(rows, te_cols):
    """rows: [NUM_CORES, n_act]; te_cols: [NUM_CORES, OW]. -> counts[NB]."""
    r1_idx, r8_idx, act_idx = _assignment()
    tot_row = rows.sum(axis=0).astype(np.float64)
    tot_te = te_cols.sum(axis=0).astype(np.float64)
    N = float(NUM_CORES * P * FTOT)
    counts = np.zeros(NB)
    for k, b in enumerate(r1_idx):
        counts[b] = tot_te[k]
    for k, b in enumerate(r8_idx):
        counts[b] = tot_te[len(r1_idx) + k]
    for i, b in enumerate(act_idx):
        counts[b] = (tot_row[i] + N) * 0.5  # sign-sum -> count
    return counts


def _ap_from_counts(counts):
    counts = np.asarray(counts, np.float32)
    fp = counts[0:T]
    tp = counts[T:2 * T]
    Pc = counts[2 * T]
    eps = np.float32(EPS)
    prec = (tp + eps) / (tp + fp + eps)
    rec = (tp + eps) / (Pc + eps)
    p = np.concatenate([[np.float32(0)], prec, [np.float32(1)]])
    r = np.concatenate([[np.float32(1)], rec, [np.float32(0)]])
    area = np.float32(0.5) * np.sum((r[1:] - r[:-1]) * (p[1:] + p[:-1]))
    return np.float32(abs(area))


def _run(y_pred, y_true, thresholds, trace=False):
    from concourse.bass_utils import run_bass_kernel_spmd

    nc = _build(thresholds)
    in_maps = _prepare_inputs(y_pred, y_true)
    last_err = None
    for attempt in range(4):
        try:
            res = run_bass_kernel_spmd(
                nc, in_maps, core_ids=list(range(NUM_CORES)), trace=trace
            )
            break
        except Exception as e:  # transient device/relay errors
            last_err = e
            import time as _time

            _time.sleep(8)
    else:
        raise last_err
    r1_idx, r8_idx, act_idx = _assignment()
    OW = CFG["onehot_w"]
    n_act = len(act_idx)
    cnts = np.stack(
        [np.asarray(res.results[i]["cnt"], np.float32).reshape(33, -1)
         for i in range(NUM_CORES)]
    )
    rows = cnts[:, 32, :n_act]
    te_cols = cnts[:, 0:OW, 0]
    counts = _decode_counts(rows, te_cols)
    out = _ap_from_counts(counts)
    return out, res


def kernel(y_pred, y_true, thresholds):
    out, _ = _run(y_pred, y_true, thresholds, trace=False)
    return out


# revision 35
# speedup vs baseline: 1.0065x; 1.0065x over previous
"""Average-Precision (histogram binning) kernel for 8 Trainium2 NeuronCores.

Reference semantics (C=2 classes, T=10 thresholds):
  s = y_pred[:, 1, ...] flattened, y = y_true flattened
  per threshold t: fp = #(y==0 & s>t), tp = #(y==1 & s>t), P = #(y==1)
  AP = trapezoid area over (recall, precision) with endpoint padding.

Device strategy (data-parallel, 1.57M voxels per core):
  Host re-encodes each (s, y) pair losslessly into one fp16 value
  v = (1-2y) * fp16(s) (label in the sign bit, score in the magnitude),
  so all 21 statistics are single-comparison counts on v:
    fp[t] = #(v > t), tp[t] = #(v < -t), P = #(v < 0).
  fp16(s) only moves each effective threshold by <= half an ulp,
  identically for tp and fp => AP error ~1e-3 << 2e-2 gate.

  Three counting lanes, balanced to the measured engine rates
  (DVE 4x tensor_scalar ~0.27ns/elem, ACT 1x ~0.85ns/elem,
   PE ones-matmul reduce ~0.5ns/elem):
   - R1: DVE tensor_scalar is_gt/is_lt makes an fp16 {0,1} mask; PE
     reduces it with a one-hot fp16 stationary into an accumulating
     PSUM row (exact integer counts).
   - R8: same mask, then one DVE tensor_tensor fold (adds the two
     halves, values {0,1,2}) so PE only reduces half the columns.
   - ACT: Sign(+-(v-theta)) with fused accum_out (sign-sum decode).
  Per-tile ACT accum columns are partition-reduced by one tiny matmul
  per tile into an accumulating [1, W] PSUM row. Host sums the 8
  per-core results, decodes, and applies the AP formula.
"""

import sys

import numpy as np

for _p in ("/opt/trn_rl_repo", "/opt/pypackages"):
    if _p not in sys.path:
        sys.path.append(_p)

NUM_CORES = 8
P = 128
FTOT = 12288  # per-core columns: 8 * 128 * 12288 = 12,582,912 voxels
EPS = 1e-7
T = 10
NB = 21  # 10 fp + 10 tp + P

CFG = {
    "tile_sizes": [2048, 6144, 4096],
    "act_idx": [15, 16, 17, 18, 19, 20],  # ACT sign lane boundaries
    "r8_idx": [10, 11, 12, 13, 14],       # DVE mask + fold -> PE half
    # remaining boundaries go to R1 (DVE mask -> PE full reduce)
    "io_bufs": 2,
    "msk_bufs": 6,
    "fld_bufs": 2,
    "sg_bufs": 2,
    "onehot_w": 16,
}


def _boundaries(thresholds):
    """21 boundaries on v: 0..9 fp[t] (gt, +t), 10..19 tp[t] (lt, -t),
    20 P (lt, 0)."""
    th = np.asarray(thresholds, np.float64)
    bounds = [("gt", float(t)) for t in th]
    bounds += [("lt", -float(t)) for t in th]
    bounds += [("lt", 0.0)]
    return bounds


def _nudge_off_grid(theta):
    """Shift theta off the fp16 grid so Sign(v - theta) never sees 0.
    Any shift smaller than half the local fp16 gap leaves all strict
    comparison counts unchanged."""
    t32 = np.float32(theta)
    if t32 == 0.0:
        return float(t32)
    if np.float32(np.float16(t32)) == t32:
        t32 = np.float32(t32 * (1.0 + 2.0 ** -12))
    return float(t32)


def _assignment():
    act_idx = CFG["act_idx"]
    r8_idx = CFG["r8_idx"]
    r1_idx = [b for b in range(NB) if b not in act_idx and b not in r8_idx]
    return r1_idx, r8_idx, act_idx


def _build(thresholds):
    from concourse import bacc, mybir
    from concourse import tile

    dt = mybir.dt
    Alu = mybir.AluOpType
    AF = mybir.ActivationFunctionType

    bounds = _boundaries(thresholds)
    sizes = CFG["tile_sizes"]
    assert sum(sizes) == FTOT
    # R8 folds reduce FT//2 columns in 512-wide PE chunks: every tile
    # size must be a multiple of 1024 or columns get silently dropped
    assert all(s % 1024 == 0 for s in sizes), sizes
    NT = len(sizes)

    r1_idx, r8_idx, act_idx = _assignment()
    n_pe = len(r1_idx) + len(r8_idx)  # PSUM rows used
    n_act = len(act_idx)
    OW = CFG["onehot_w"]
    assert n_pe <= OW

    nc = bacc.Bacc(
        "TRN2", target_bir_lowering=False, debug=False, num_devices=NUM_CORES
    )
    v_exts = [
        nc.dram_tensor(f"v{j}", [P, sizes[j]], dt.float16,
                       kind="ExternalInput")
        for j in range(NT)
    ]
    # single merged output: rows 0..OW-1 col0 = PE counts, row OW = sign sums
    cnt_ext = nc.dram_tensor("cnt", [33, max(n_act, 1)], dt.float32,
                             kind="ExternalOutput")

    with tile.TileContext(nc) as tc:
        with (
            tc.tile_pool(name="io", bufs=CFG["io_bufs"]) as io_pool,
            tc.tile_pool(name="msk", bufs=CFG["msk_bufs"]) as msk_pool,
            tc.tile_pool(name="fld", bufs=CFG["fld_bufs"]) as fld_pool,
            tc.tile_pool(name="sg", bufs=CFG["sg_bufs"]) as sg_pool,
            tc.tile_pool(name="acc", bufs=2) as acc_pool,
            tc.tile_pool(name="fin", bufs=1) as fin_pool,
            tc.tile_pool(name="cst", bufs=1) as cst_pool,
            tc.tile_pool(name="psA", bufs=1, space="PSUM") as psA_pool,
            tc.tile_pool(name="psB", bufs=1, space="PSUM") as psB_pool,
        ):
            # ---- input DMAs first: get bytes moving before const setup.
            # All on the Sync HWDGE ring: serialized transfers give tile 0
            # the full bandwidth, minimizing time-to-first-compute (issuing
            # tile 0 concurrently from the Scalar sequencer was measured to
            # split bandwidth and delay tile 0 by ~3us). Each tile is its
            # own contiguous DRAM tensor so the transfer is fully dense.
            v_tiles = []
            for j in range(NT):
                FT = sizes[j]
                v_t = io_pool.tile([P, FT], dt.float16, tag="v", name=f"v_{j}")
                nc.sync.dma_start(out=v_t[:], in_=v_exts[j][:, :])
                v_tiles.append(v_t)

            # ---- constants ----
            # one-hot stationary blocks: block i is [P, OW] with column i
            # all-ones, so PE lands boundary i's mask-sum in PSUM row i.
            oh = cst_pool.tile([P, n_pe * OW], dt.float16, name="oh")
            nc.vector.memset(oh[:], 0.0)
            for i in range(n_pe):
                nc.vector.memset(oh[:, i * OW + i : i * OW + i + 1], 1.0)
            ones_f32 = cst_pool.tile([P, 1], dt.float32, name="ones_f32")
            nc.vector.memset(ones_f32[:], 1.0)
            act_bias = []
            for i, b in enumerate(act_idx):
                kind, thr = bounds[b]
                theta = _nudge_off_grid(thr)
                bias = cst_pool.tile([P, 1], dt.float32, name=f"abias_{i}")
                # gt: sign(v - theta); lt: sign(theta - v)
                nc.vector.memset(bias[:], -theta if kind == "gt" else theta)
                act_bias.append(bias)

            # warm the Sign table set while the first DMA is in flight so
            # the ~2.7us ACT_TABLE_LOAD+drain stays off the ACT lane
            warm = cst_pool.tile([P, 1], dt.float16, name="warm")
            nc.scalar.activation(out=warm[:], in_=ones_f32[:], func=AF.Sign,
                                 bias=act_bias[0][:], scale=1.0)

            ps_te = psA_pool.tile([OW, 512], dt.float32, name="ps_te")
            ps_row = psB_pool.tile([1, n_act], dt.float32, name="ps_row")
            first_mm = [True]

            acc_tiles = []
            for j in range(NT):
                FT = sizes[j]
                last_tile = j == NT - 1
                v_t = v_tiles[j]

                # ---- ACT lane: Sign + fused accum ----
                acc_t = acc_pool.tile([P, n_act], dt.float32, tag="acc",
                                      name=f"acc_{j}")
                acc_tiles.append(acc_t)
                for i, b in enumerate(act_idx):
                    kind, _ = bounds[b]
                    scl = 1.0 if kind == "gt" else -1.0
                    sg = sg_pool.tile([P, FT], dt.float16, tag="sg",
                                      name=f"sg_{j}_{i}")
                    nc.scalar.activation(out=sg[:], in_=v_t[:], func=AF.Sign,
                                         bias=act_bias[i][:], scale=scl,
                                         accum_out=acc_t[:, i:i + 1])

                # ---- R1/R8 lanes interleaved (smooth PE feed):
                # R1: DVE mask -> PE full-width reduce
                # R8: DVE mask -> fold halves -> PE half reduce
                seq = []
                it1, it8 = iter(enumerate(r1_idx)), iter(enumerate(r8_idx))
                for t in range(max(len(r1_idx), len(r8_idx))):
                    for it, lane in ((it1, "r1"), (it1, "r1"), (it8, "r8")):
                        nxt = next(it, None)
                        if nxt is not None:
                            seq.append((lane, nxt[0], nxt[1]))
                n_seq = len(seq)
                for si, (lane, k, b) in enumerate(seq):
                    # previous tile's sign-sum partition-reduce lands a few
                    # MMs into this tile, so PE never stalls waiting on ACT
                    # at the tile boundary
                    if j > 0 and si == 2:
                        nc.tensor.matmul(ps_row[:], ones_f32[:],
                                         acc_tiles[j - 1][:],
                                         start=(j == 1), stop=False)
                    kind, thr = bounds[b]
                    op = Alu.is_gt if kind == "gt" else Alu.is_lt
                    mk = msk_pool.tile([P, FT], dt.float16, tag="mk",
                                       name=f"mk_{j}_{lane}_{k}")
                    nc.vector.tensor_scalar(out=mk[:], in0=v_t[:],
                                            scalar1=thr, scalar2=None, op0=op)
                    # fractional rebalance hook: folding an R1 boundary on a
                    # subset of tiles lands in the same PSUM row (counts add
                    # exactly); measured engine rates put the optimum at the
                    # plain 10/5 split, so no override is active
                    fold = lane == "r8"
                    if not fold:
                        red, kk = mk, k
                    else:
                        H = FT // 2
                        fd = fld_pool.tile([P, H], dt.float16, tag="fd",
                                           name=f"fd_{j}_{lane}_{k}")
                        nc.vector.tensor_tensor(out=fd[:], in0=mk[:, 0:H],
                                                in1=mk[:, H:FT], op=Alu.add)
                        kk = k if lane == "r1" else len(r1_idx) + k
                        red = fd
                    W = FT if not fold else FT // 2
                    for c in range(W // 512):
                        nc.tensor.matmul(
                            ps_te[:],
                            oh[:, kk * OW : kk * OW + OW],
                            red[:, c * 512:(c + 1) * 512],
                            start=first_mm[0],
                            stop=(last_tile and si == n_seq - 1
                                  and c == W // 512 - 1),
                        )
                        first_mm[0] = False

            # ---- last tile's sign-sum partition-reduce ----
            nc.tensor.matmul(ps_row[:], ones_f32[:], acc_tiles[NT - 1][:],
                             start=False, stop=True)

            # ---- finalize (reduce/copy straight from PSUM, one out DMA) ----
            fin = fin_pool.tile([33, max(n_act, 1)], dt.float32, name="fin")
            nc.vector.tensor_reduce(out=fin[0:OW, 0:1], in_=ps_te[:],
                                    axis=mybir.AxisListType.X, op=Alu.add)
            nc.vector.tensor_copy(fin[32:33, 0:n_act], ps_row[:])
            nc.sync.dma_start(out=cnt_ext[:], in_=fin[:])

    nc.compile()
    return nc


def _prepare_inputs(y_pred, y_true):
    """v = (1-2y) * fp16(s): lossless per-voxel re-encode of (s, y).
    Split per compute tile into contiguous blocks for dense DMA."""
    s = np.asarray(y_pred)[:, 1].reshape(-1).astype(np.float16)
    y = np.asarray(y_true).reshape(-1)
    v = np.where(y == 0, s, -s)
    n = v.size
    assert n == NUM_CORES * P * FTOT, n
    v_sh = v.reshape(NUM_CORES, P, FTOT)
    sizes = CFG["tile_sizes"]
    maps = []
    for i in range(NUM_CORES):
        m, col0 = {}, 0
        for j, ft in enumerate(sizes):
            m[f"v{j}"] = np.ascontiguousarray(v_sh[i, :, col0:col0 + ft])
            col0 += ft
        maps.append(m)
    return maps


def _decode_counts(rows, te_cols):
    """rows: [NUM_CORES, n_act]; te_cols: [NUM_CORES, OW]. -> counts[NB]."""
    r1_idx, r8_idx, act_idx = _assignment()
    tot_row = rows.sum(axis=0).astype(np.float64)
    tot_te = te_cols.sum(axis=0).astype(np.float64)
    N = float(NUM_CORES * P * FTOT)
    counts = np.zeros(NB)
    for k, b in enumerate(r1_idx):
        counts[b] = tot_te[k]
    for k, b in enumerate(r8_idx):
        counts[b] = tot_te[len(r1_idx) + k]
    for i, b in enumerate(act_idx):
        counts[b] = (tot_row[i] + N) * 0.5  # sign-sum -> count
    return counts


def _ap_from_counts(counts):
    counts = np.asarray(counts, np.float32)
    fp = counts[0:T]
    tp = counts[T:2 * T]
    Pc = counts[2 * T]
    eps = np.float32(EPS)
    prec = (tp + eps) / (tp + fp + eps)
    rec = (tp + eps) / (Pc + eps)
    p = np.concatenate([[np.float32(0)], prec, [np.float32(1)]])
    r = np.concatenate([[np.float32(1)], rec, [np.float32(0)]])
    area = np.float32(0.5) * np.sum((r[1:] - r[:-1]) * (p[1:] + p[:-1]))
    return np.float32(abs(area))


def _run(y_pred, y_true, thresholds, trace=False):
    from concourse.bass_utils import run_bass_kernel_spmd

    nc = _build(thresholds)
    in_maps = _prepare_inputs(y_pred, y_true)
    last_err = None
    for attempt in range(4):
        try:
            res = run_bass_kernel_spmd(
                nc, in_maps, core_ids=list(range(NUM_CORES)), trace=trace
            )
            break
        except Exception as e:  # transient device/relay errors
            last_err = e
            import time as _time

            _time.sleep(8)
    else:
        raise last_err
    r1_idx, r8_idx, act_idx = _assignment()
    OW = CFG["onehot_w"]
    n_act = len(act_idx)
    cnts = np.stack(
        [np.asarray(res.results[i]["cnt"], np.float32).reshape(33, -1)
         for i in range(NUM_CORES)]
    )
    rows = cnts[:, 32, :n_act]
    te_cols = cnts[:, 0:OW, 0]
    counts = _decode_counts(rows, te_cols)
    out = _ap_from_counts(counts)
    return out, res


def kernel(y_pred, y_true, thresholds):
    out, _ = _run(y_pred, y_true, thresholds, trace=False)
    return out
